# revision 14
# baseline (speedup 1.0000x reference)
"""Trainium2 Bass kernel for nn_AttentionBlock (GroupNorm + 8-head self-attention + residual).

Full inputs in, full output out. Sharding: data-parallel over batch across the
8 NeuronCores (16 batches -> 2 per core), weights replicated, no collectives.

Layout strategy (per core, per batch; C=512 channels, S=1024 tokens):
  - x and xhat live as [C, S] tiles (channels on partitions); GroupNorm
    group stats via tiny PE matmuls against one-hot group matrices; the
    xhat normalization runs on the gpsimd (Pool) engine to keep DVE free.
  - Q^T, K^T computed as [qk_rows, S] (head-major rows, bf16); K is stored as
    even/odd zero-padded tiles so the scores matmul contracts over all 128
    partitions (64-partition contraction streams at half rate on TRN2).
  - scores are computed TRANSPOSED: scoresT[j, i] = k_j . q_i so the softmax
    reduction (over j) aligns with the PV contraction and no transposes are
    needed.  exp(scores - 1) is computed from PSUM by the Scalar engine
    (fp8e4 out) for most j-blocks and by a DVE int16 Schraudolph bit-trick
    (bf16 out) for a configurable subset, balancing the two engines.
  - V tiles are fp8e4 (weights scaled x4 host-side) with a ones-column per
    head; P@V runs as fp8 DoubleRow matmuls (2 j-blocks per instruction,
    2x FLOP rate); bf16-exp j-block pairs fall back to two plain matmuls
    (fp8 stationary x bf16 moving is supported).
  - softmax normalization: Scalar engine copies the [65, 512] PV PSUM block
    to SBUF; gpsimd computes 1/rowsum via the int32 magic-constant trick
    (~4% err, attenuated ~20x by the residual), broadcasts it, and applies
    it writing fp8 resT tiles (values = 4*attn, in e4m3 sweet spot).
  - out-projection: fp8 DoubleRow (wo scaled x4 host-side; net 16x undone in
    the fused epilogue scale+residual-add on DVE).
Irreducible floors: scores matmuls (PE, 27us/batch), exp (~8.4M elems/batch
across ACT+DVE).  The schedule interleaves projection/epilogue work into the
attention loops so the Tensor engine never idles while ScalarE/DVE exp.
"""

import numpy as np
import ml_dtypes

import concourse.bacc as bacc
import concourse.tile as tile
from concourse import mybir
from concourse.bass_utils import run_bass_kernel_spmd

N_CORES = 8
B, C, H, W = 16, 512, 32, 32
S = H * W                      # 1024
BL = B // N_CORES              # 2 batches per core
NH, DK = 8, 64
NG = 32                        # groupnorm groups
GSZ = C // NG                  # 16 channels per group
EPS = 1e-5
F32 = mybir.dt.float32
BF16 = mybir.dt.bfloat16
F8 = mybir.dt.float8e4
I16 = mybir.dt.int16
I32 = mybir.dt.int32
AF = mybir.ActivationFunctionType
OP = mybir.AluOpType
PM = mybir.MatmulPerfMode
NPBF16 = ml_dtypes.bfloat16
NPF8 = ml_dtypes.float8_e4m3

# Schraudolph exp bit-trick constants (bf16/int16 space), with the softmax
# bias exp(x-1) folded in (cancels in the normalization).
EXP_A16 = 128.0 / np.log(2.0)                 # 184.665
EXP_B16 = 127.0 * 128 - 486411.0 / 65536.0 - EXP_A16   # mean-centered, bias -1
# int32 magic reciprocal constant (~4% max rel err)
RECIP_C = 0x7EF311C3

# jb-pairs (0..3) whose exp runs on DVE (bf16) instead of ACT (fp8), per
# (batch, head).  Batch 1 leans harder on DVE: its fill work (projections,
# copies) is mostly done by then, while ACT exp otherwise gates the tail.
DVE_PAIRS = {}
for _h in range(NH):
    DVE_PAIRS[(0, _h)] = (1,) if _h % 2 == 0 else ()
    DVE_PAIRS[(1, _h)] = (1,) if _h % 2 == 0 else (3,)

# test.py can flip these; results stashed in LAST.
TRACE = False
LAST = {}


def _build():
    nc = bacc.Bacc()

    x_d = nc.dram_tensor("x", [BL, C, S], F32, kind="ExternalInput")
    wqt_d = nc.dram_tensor("wqt", [C, C], BF16, kind="ExternalInput")   # [c_in, q_row]
    wkt_d = nc.dram_tensor("wkt", [C, C], BF16, kind="ExternalInput")
    wvt_d = nc.dram_tensor("wvt", [C, C], BF16, kind="ExternalInput")
    wop_d = nc.dram_tensor("wop", [2, 128, 2, C], F8, kind="ExternalInput")
    g_d = nc.dram_tensor("gmat", [128, 8], F32, kind="ExternalInput")
    gt_d = nc.dram_tensor("gtmat", [8, 128], F32, kind="ExternalInput")
    zpad_d = nc.dram_tensor("zpad", [64, S], BF16, kind="ExternalInput")
    out_d = nc.dram_tensor("out", [BL, C, S], F32, kind="ExternalOutput")

    with tile.TileContext(nc) as tc:
        with (
            tc.tile_pool(name="const", bufs=1) as const,
            tc.tile_pool(name="px", bufs=4) as px,
            tc.tile_pool(name="pxe", bufs=4) as pxe,
            tc.tile_pool(name="pgn", bufs=4) as pgn,
            tc.tile_pool(name="pxh", bufs=8) as pxh,
            tc.tile_pool(name="pqt", bufs=8) as pqt,
            tc.tile_pool(name="pkt", bufs=16) as pkt,
            tc.tile_pool(name="pv", bufs=8) as pvp,
            tc.tile_pool(name="pexp", bufs=6) as pexp,
            tc.tile_pool(name="prr", bufs=4) as prr,
            tc.tile_pool(name="prt", bufs=4) as prt,
            tc.tile_pool(name="pout", bufs=3) as pout,
            tc.tile_pool(name="pps", bufs=2, space="PSUM") as pps,
            tc.tile_pool(name="psc", bufs=2, space="PSUM") as psc,
            tc.tile_pool(name="ppv", bufs=2, space="PSUM") as ppv,
        ):
            # ---- batch-0 x first (groupnorm needs it before weights)
            xt0 = []
            for cb in range(4):
                t = px.tile([128, S], F32, tag="x", name=f"x0_{cb}")
                nc.sync.dma_start(out=t, in_=x_d[0, cb * 128 : (cb + 1) * 128, :])
                xt0.append(t)

            # ---- constants into SBUF
            g_sb = const.tile([128, 8], F32, tag="g")
            nc.sync.dma_start(out=g_sb, in_=g_d[:, :])
            gt_sb = const.tile([8, 128], F32, tag="gt")
            nc.sync.dma_start(out=gt_sb, in_=gt_d[:, :])
            wq_sb, wk_sb, wv_sb = [], [], []
            for nm, lst, srcd in (("q", wq_sb, wqt_d), ("k", wk_sb, wkt_d), ("v", wv_sb, wvt_d)):
                for cb in range(4):
                    t = const.tile([128, C], BF16, tag=f"w_{nm}_{cb}")
                    nc.sync.dma_start(out=t, in_=srcd[cb * 128 : (cb + 1) * 128, :])
                    lst.append(t)
            wo_sb = []
            for a in range(2):
                t = const.tile([128, 2, C], F8, tag=f"w_o_{a}")
                nc.sync.dma_start(out=t, in_=wop_d[a])
                wo_sb.append(t)
            nbias = const.tile([128, 1], F32, tag="nbias")
            nc.vector.memset(nbias, -1.0)

            # ---- PE warm-up during the DMA/groupnorm-bound startup so the
            # HAM clock gate opens before the first real projection matmul.
            warm_ps = pps.tile([8, 128], F32, tag="pp", name="warm_ps")
            for wi in range(12):
                nc.tensor.matmul(
                    out=warm_ps, lhsT=g_sb, rhs=xt0[0][:, 0:128], start=True, stop=True
                )

            # ================= emission helpers =================
            def load_x(b):
                xt = []
                for cb in range(4):
                    t = px.tile([128, S], F32, tag="x", name=f"x{b}_{cb}")
                    nc.sync.dma_start(out=t, in_=x_d[b, cb * 128 : (cb + 1) * 128, :])
                    xt.append(t)
                return xt

            def gn_batch(b, xt, xh):
                # groupnorm stats + xhat on DVE (gpsimd ALU is ~14x slower/elem)
                pgall = pps.tile([8, 4, 2], F32, tag="pp")   # [group, cb, (mean,e2)]
                for cb in range(4):
                    st6 = pgn.tile([128, 2, 6], F32, tag="st6")
                    nc.vector.bn_stats(out=st6[:, 0, :], in_=xt[cb][:, 0:512])
                    nc.vector.bn_stats(out=st6[:, 1, :], in_=xt[cb][:, 512:1024])
                    mv = pgn.tile([128, 2], F32, tag="mv")
                    nc.vector.bn_aggr(out=mv, in_=st6)
                    me2 = pgn.tile([128, 2], F32, tag="me2")
                    nc.vector.tensor_copy(out=me2[:, 0:1], in_=mv[:, 0:1])
                    nc.vector.tensor_tensor(
                        out=me2[:, 1:2], in0=mv[:, 0:1], in1=mv[:, 0:1], op=OP.mult
                    )
                    nc.vector.tensor_tensor(
                        out=me2[:, 1:2], in0=me2[:, 1:2], in1=mv[:, 1:2], op=OP.add
                    )
                    nc.tensor.matmul(
                        out=pgall[:, cb, :], lhsT=g_sb, rhs=me2, start=True, stop=True
                    )
                # group stats for all blocks at once ([8, 4] tiles)
                gm = pgn.tile([8, 4], F32, tag="gm")
                z = pgn.tile([8, 4], F32, tag="z")
                t2 = pgn.tile([8, 4], F32, tag="t2")
                y = pgn.tile([8, 4], F32, tag="y")
                nc.vector.tensor_scalar(
                    out=gm, in0=pgall[:, :, 0], scalar1=1.0 / GSZ, scalar2=None,
                    op0=OP.mult,
                )
                nc.vector.tensor_scalar(
                    out=z, in0=pgall[:, :, 1], scalar1=1.0 / GSZ, scalar2=EPS,
                    op0=OP.mult, op1=OP.add,
                )
                nc.vector.tensor_tensor(out=t2, in0=gm, in1=gm, op=OP.mult)
                nc.vector.tensor_tensor(out=z, in0=z, in1=t2, op=OP.subtract)
                # rsqrt(z): y0 = 1/z, then y <- y*(1.5 - 0.5*z*y^2) twice
                nc.vector.reciprocal(out=y, in_=z)
                for _ in range(2):
                    nc.vector.tensor_tensor(out=t2, in0=z, in1=y, op=OP.mult)
                    nc.vector.tensor_tensor(out=t2, in0=t2, in1=y, op=OP.mult)
                    nc.vector.tensor_scalar(
                        out=t2, in0=t2, scalar1=-0.5, scalar2=1.5,
                        op0=OP.mult, op1=OP.add,
                    )
                    nc.vector.tensor_tensor(out=y, in0=y, in1=t2, op=OP.mult)
                gs2 = pgn.tile([8, 2, 4], F32, tag="gs2")   # [(mean,rstd), cb]
                nc.vector.tensor_copy(out=gs2[:, 0, :], in_=gm)
                nc.vector.tensor_copy(out=gs2[:, 1, :], in_=y)
                for cb in range(4):
                    pb = pps.tile([128, 2], F32, tag="pp")
                    nc.tensor.matmul(
                        out=pb, lhsT=gt_sb, rhs=gs2[:, :, cb], start=True, stop=True
                    )
                    t = pxh.tile([128, S], BF16, tag="xh", name=f"xh{b}_{cb}")
                    nc.vector.tensor_scalar(
                        out=t,
                        in0=xt[cb],
                        scalar1=pb[:, 0:1],
                        scalar2=pb[:, 1:2],
                        op0=OP.subtract,
                        op1=OP.mult,
                    )
                    xh.append(t)

            def v_group(b, xh, vt, st):
                # V pair-tiles [128, 2, NH, 65] fp8; slot = st%2; ones col = 1.0
                pv = pps.tile([128, 512], F32, tag="pp")
                for cb in range(4):
                    nc.tensor.matmul(
                        out=pv,
                        lhsT=xh[cb][:, st * 128 : (st + 1) * 128],
                        rhs=wv_sb[cb],
                        start=(cb == 0),
                        stop=(cb == 3),
                    )
                if st % 2 == 0:
                    t = pvp.tile([128, 2, NH, 72], F8, tag="v", name=f"v{b}_{st // 2}")
                    vt.append(t)
                else:
                    t = vt[st // 2]
                sl = st % 2
                nc.vector.memset(t[:, sl, :, 64:65], 1.0)
                nc.scalar.activation(
                    out=t[:, sl, :, 0:64],
                    in_=pv.rearrange("p (h d) -> p h d", h=NH),
                    func=AF.Copy,
                )

            def attn_head(b, qt, kt, vt, rt, h, fill=None, fill_every=2):
                # scoresT -> exp (ACT fp8 / DVE bf16 bit-trick) -> P@V fp8
                # DoubleRow pairs -> gpsimd softmax-normalize into fp8 resT.
                hp = h // 2
                dve_pairs = DVE_PAIRS[(b, h)]
                ex_tiles = [None] * 4          # (tile, is_fp8) per jb-pair
                for jb in range(8):
                    ps = psc.tile([128, S], F32, tag="ps")
                    for sc in range(2):
                        cols = slice(sc * 512, (sc + 1) * 512)
                        nc.tensor.matmul(
                            out=ps[:, cols],
                            lhsT=kt[h][:, jb * 128 : (jb + 1) * 128],
                            rhs=qt[hp][:, cols],
                            start=True,
                            stop=True,
                        )
                    pair, sl = jb // 2, jb % 2
                    if sl == 0:
                        if pair in dve_pairs:
                            ex = pexp.tile([128, 2, S], I16, tag="exb", name=f"exb{b}_{h}_{pair}")
                            ex_tiles[pair] = (ex, False)
                        else:
                            ex = pexp.tile([128, 2, S], F8, tag="exf", name=f"exf{b}_{h}_{pair}")
                            ex_tiles[pair] = (ex, True)
                    ex, is_f8 = ex_tiles[pair]
                    if is_f8:
                        nc.scalar.activation(
                            out=ex[:, sl, :], in_=ps, func=AF.Exp, bias=nbias[:, 0:1]
                        )
                    else:
                        nc.vector.tensor_scalar(
                            out=ex[:, sl, :], in0=ps,
                            scalar1=EXP_A16, scalar2=EXP_B16,
                            op0=OP.mult, op1=OP.add,
                        )
                    if fill is not None and jb % fill_every == fill_every - 1:
                        fill()

                # P@V: fp8 pairs as DoubleRow; bf16 pairs as 2 plain matmuls
                n_mm = sum(1 if f8 else 2 for _, f8 in ex_tiles)
                a, loc = h // 4, h % 4
                rsl, rrow = loc // 2, (loc % 2) * 64
                for sc in range(2):
                    cols = slice(sc * 512, (sc + 1) * 512)
                    pvt = ppv.tile([65, 512], F32, tag="ppvt", name=f"pvt{b}_{h}_{sc}")
                    mi = 0
                    for pair in range(4):
                        ex, is_f8 = ex_tiles[pair]
                        if is_f8:
                            nc.tensor.matmul(
                                out=pvt,
                                lhsT=vt[pair][:, :, h, 0:65],
                                rhs=ex[:, :, cols],
                                start=(mi == 0),
                                stop=(mi == n_mm - 1),
                                perf_mode=PM.DoubleRow,
                            )
                            mi += 1
                        else:
                            exb = ex.bitcast(BF16)
                            for sl in range(2):
                                nc.tensor.matmul(
                                    out=pvt,
                                    lhsT=vt[pair][:, sl, h, 0:65],
                                    rhs=exb[:, sl, cols],
                                    start=(mi == 0),
                                    stop=(mi == n_mm - 1),
                                )
                                mi += 1
                    # normalize: rt[a][rows, slot, cols] = pvt[0:64] / sums
                    # (fp8 out).  recip: int32 magic trick on DVE straight from
                    # PSUM; bcast on gpsimd (its only op type -> no lib thrash)
                    r0 = prr.tile([1, 512], I32, tag="r0")
                    nc.vector.tensor_scalar(
                        out=r0, in0=pvt[64:65, :].bitcast(I32),
                        scalar1=-1, scalar2=RECIP_C, op0=OP.mult, op1=OP.add,
                    )
                    rbt = prr.tile([64, 512], F32, tag="rb")
                    nc.gpsimd.partition_broadcast(rbt, r0.bitcast(F32))
                    nc.vector.tensor_tensor(
                        out=rt[a][rrow : rrow + 64, rsl, cols],
                        in0=pvt[0:64, :],
                        in1=rbt,
                        op=OP.mult,
                    )

            def drain(wl, n):
                for _ in range(min(n, len(wl))):
                    wl.pop(0)()

            def qk_units(b, xh, dst, w_sb, rb):
                # one projection psum row-block.  For Q: a single head-pair
                # tile.  For K: two per-head tiles with the other head's 64
                # rows zeroed (full 128-partition contraction in scores).
                holder = {}
                is_q = w_sb is wq_sb

                def half(sc):
                    if "t" not in holder:
                        if is_q:
                            tq = pqt.tile([128, S], BF16, tag="qk", name=f"q{b}_{rb}")
                            holder["t"] = (tq,)
                            dst.append(tq)
                        else:
                            te = pkt.tile([128, S], BF16, tag="qk", name=f"k{b}_{rb}e")
                            to = pkt.tile([128, S], BF16, tag="qk", name=f"k{b}_{rb}o")
                            nc.sync.dma_start(out=te[64:128, :], in_=zpad_d[:, :])
                            nc.sync.dma_start(out=to[0:64, :], in_=zpad_d[:, :])
                            holder["t"] = (te, to)
                            dst.extend([te, to])
                    tiles = holder["t"]
                    pq = pps.tile([128, 512], F32, tag="pp")
                    for cb in range(4):
                        nc.tensor.matmul(
                            out=pq,
                            lhsT=w_sb[cb][:, rb * 128 : (rb + 1) * 128],
                            rhs=xh[cb][:, sc * 512 : (sc + 1) * 512],
                            start=(cb == 0),
                            stop=(cb == 3),
                        )
                    cols = slice(sc * 512, (sc + 1) * 512)
                    if is_q:
                        nc.vector.tensor_copy(out=tiles[0][:, cols], in_=pq)
                    else:
                        for t, prng in (
                            (tiles[0], slice(0, 64)),
                            (tiles[1], slice(64, 128)),
                        ):
                            nc.vector.tensor_copy(out=t[prng, cols], in_=pq[prng, :])

                return [lambda: half(0), lambda: half(1)]

            def epi_units(b, rt, cb):
                # out-projection fp8 DoubleRow + fused 1/16 scale + residual
                holder = {}

                def half(sc):
                    if "t" not in holder:
                        holder["t"] = pout.tile([128, S], F32, tag="ot", name=f"ot{b}_{cb}")
                    ot = holder["t"]
                    xre = pxe.tile([128, 512], F32, tag="xe")
                    nc.sync.dma_start(
                        out=xre,
                        in_=x_d[b, cb * 128 : (cb + 1) * 128, sc * 512 : (sc + 1) * 512],
                    )
                    po = pps.tile([128, 512], F32, tag="pp")
                    for a in range(2):
                        nc.tensor.matmul(
                            out=po,
                            lhsT=wo_sb[a][:, :, cb * 128 : (cb + 1) * 128],
                            rhs=rt[a][:, :, sc * 512 : (sc + 1) * 512],
                            start=(a == 0),
                            stop=(a == 1),
                            perf_mode=PM.DoubleRow,
                        )
                    dst_ap = ot[:, sc * 512 : (sc + 1) * 512]
                    nc.vector.scalar_tensor_tensor(
                        out=dst_ap,
                        in0=po,
                        scalar=1.0 / 16.0,
                        in1=xre,
                        op0=OP.mult,
                        op1=OP.add,
                    )
                    if sc == 1:
                        nc.sync.dma_start(
                            out=out_d[b, cb * 128 : (cb + 1) * 128, :], in_=ot
                        )

                return [lambda: half(0), lambda: half(1)]

            # ================= schedule =================
            xt1 = load_x(1)
            xh0, qt0, kt0, vt0 = [], [], [], []
            gn_batch(0, xt0, xh0)
            for u in qk_units(0, xh0, qt0, wq_sb, 0):
                u()
            for u in qk_units(0, xh0, kt0, wk_sb, 0):
                u()

            xh1, qt1, kt1, vt1 = [], [], [], []
            gn_batch(1, xt1, xh1)
            for u in qk_units(1, xh1, qt1, wq_sb, 0):
                u()
            for u in qk_units(1, xh1, kt1, wk_sb, 0):
                u()

            # Batch-interleaved head schedule (b1 lags b0 by two heads) keeps
            # the exp/normalize load on ACT/DVE evenly mixed with PE work over
            # the whole run instead of leaving an elementwise-paced tail.
            # Per-batch fill queues: V groups first (head 0 drains one per jb,
            # fill_every=1, so all 4 V pair-tiles exist by its P@V block),
            # then the remaining projection row-blocks.
            workq = {
                b: [lambda st=st, b=b: v_group(b, (xh0, xh1)[b], (vt0, vt1)[b], st)
                    for st in range(8)]
                for b in range(2)
            }
            for rb in range(1, 4):
                workq[0].extend(qk_units(0, xh0, qt0, wq_sb, rb))
                workq[0].extend(qk_units(0, xh0, kt0, wk_sb, rb))
                workq[1].extend(qk_units(1, xh1, qt1, wq_sb, rb))
                workq[1].extend(qk_units(1, xh1, kt1, wk_sb, rb))

            rt0 = [prt.tile([128, 2, S], F8, tag="rt", name=f"rt0_{i}") for i in range(2)]
            rt1 = [prt.tile([128, 2, S], F8, tag="rt", name=f"rt1_{i}") for i in range(2)]
            rts = (rt0, rt1)
            qts, kts, vts = (qt0, qt1), (kt0, kt1), (vt0, vt1)
            seq = [(0, 0), (0, 1)]
            for h in range(6):
                seq += [(1, h), (0, h + 2)]
            seq += [(1, 6), (1, 7)]
            nheads_done = {0: 0, 1: 0}
            for b, h in seq:
                wl = workq[b]
                attn_head(
                    b, qts[b], kts[b], vts[b], rts[b], h,
                    fill=lambda wl=wl: drain(wl, 1),
                    fill_every=(1 if h == 0 else 2),
                )
                nheads_done[b] += 1
                if b == 0 and h == 7:
                    # batch-0 epilogue drains during batch-1's last heads
                    for cb in range(4):
                        workq[1].extend(epi_units(0, rt0, cb))
            drain(workq[0], len(workq[0]))
            drain(workq[1], len(workq[1]))
            for cb in range(4):
                for u in epi_units(1, rt1, cb):
                    u()

    nc.finalize()
    return nc


def kernel(**inputs):
    x = np.asarray(inputs["x"], np.float32)
    norm_w = np.asarray(inputs["norm_w"], np.float64)
    norm_b = np.asarray(inputs["norm_b"], np.float64)
    proj_w = np.asarray(inputs["proj_w"], np.float64)
    proj_b = np.asarray(inputs["proj_b"], np.float64)
    out_w = np.asarray(inputs["out_w"], np.float32)
    out_b = np.asarray(inputs["out_b"], np.float32)

    # split qkv rows (row = h*192 + t*64 + d, t in {q,k,v}) into head-major mats
    pw = proj_w.reshape(NH, 3, DK, C)
    pb = proj_b.reshape(NH, 3, DK)
    mats, biases = [], []
    for t in range(3):
        wm = pw[:, t].reshape(NH * DK, C)
        bv = pb[:, t].reshape(NH * DK)
        # fold groupnorm affine: y = xhat*nw + nb  =>  W@y + b = (W*nw)@xhat + (W@nb + b)
        mats.append(wm * norm_w[None, :])
        biases.append(bv + wm @ norm_b)
    wq, wk, wv = mats
    bq, bk, bv = biases
    if np.any(bq) or np.any(bk) or np.any(bv) or np.any(out_b):
        raise NotImplementedError("nonzero qkv/out biases not supported")
    scale = DK ** -0.5
    wq = wq * scale

    wqT = np.ascontiguousarray(wq.T).astype(NPBF16)
    wkT = np.ascontiguousarray(wk.T).astype(NPBF16)
    wvT = np.ascontiguousarray(wv.T * 4.0).astype(NPBF16)
    # out-proj in fp8 DoubleRow pair layout: wop[a][p, s, c] = 4*out_w.T[a*256+s*128+p, c]
    woT = np.ascontiguousarray(out_w.T) * 4.0
    wop = np.ascontiguousarray(
        woT.reshape(2, 2, 128, C).transpose(0, 2, 1, 3)
    ).astype(NPF8)

    G = np.zeros((128, 8), np.float32)
    G[np.arange(128), np.arange(128) // GSZ] = 1.0
    GT = np.ascontiguousarray(G.T)
    ZPAD = np.zeros((64, S), NPBF16)

    nc = _build()

    xr = x.reshape(B, C, S)
    in_maps = []
    for c in range(N_CORES):
        in_maps.append({
            "x": np.ascontiguousarray(xr[c * BL : (c + 1) * BL]),
            "wqt": wqT,
            "wkt": wkT,
            "wvt": wvT,
            "wop": wop,
            "gmat": G,
            "gtmat": GT,
            "zpad": ZPAD,
        })

    # guard: bass_utils imports antenv.axon_hooks when tracing is requested
    # (e.g. via BASS_TRACE env); provide a no-op module if the image lacks it.
    try:
        import antenv.axon_hooks  # noqa: F401
    except ImportError:
        import sys
        import types

        import antenv

        _m = types.ModuleType("antenv.axon_hooks")
        _m._hook = None
        _m.set_axon_ntff_profile_hook = lambda h: setattr(_m, "_hook", h)
        _m.get_axon_ntff_profile_hook = lambda: _m._hook
        sys.modules["antenv.axon_hooks"] = _m
        antenv.axon_hooks = _m

    res = None
    for attempt in range(3):
        try:
            res = run_bass_kernel_spmd(
                nc, in_maps, core_ids=list(range(N_CORES)), trace=TRACE
            )
            break
        except Exception:
            # transient NRT_EXEC_UNIT_UNRECOVERABLE-style device hiccups
            # clear on retry; re-raise on the final attempt
            if attempt == 2:
                raise
    LAST["exec_time_ns"] = res.exec_time_ns
    LAST["mean_exec_time_ns"] = res.mean_exec_time_ns
    LAST["result"] = res

    out = np.concatenate([res.results[c]["out"] for c in range(N_CORES)], axis=0)
    return np.ascontiguousarray(out.reshape(B, C, H, W).astype(np.float32))


# revision 15
# speedup vs baseline: 1.2983x; 1.2983x over previous
"""Trainium2 Bass kernel for nn_AttentionBlock (GroupNorm + 8-head self-attention + residual).

Full inputs in, full output out. Sharding: data-parallel over batch across the
8 NeuronCores (16 batches -> 2 per core), weights replicated, no collectives.

Layout strategy (per core, per batch; C=512 channels, S=1024 tokens):
  - x and xhat live as [C, S] tiles (channels on partitions) so GroupNorm
    scale/bias are per-partition scalars.  Cross-partition group reductions
    (16 channels/group) and the broadcast back are tiny PE matmuls against
    one-hot group matrices.
  - Q^T, K^T computed as [qk_rows, S] (head-major rows); V as [S, 8*65] with a
    ones-column per head (row-sums of exp fall out of the P@V matmul).
  - scores are computed TRANSPOSED: scoresT[j, i] = k_j . q_i so that the
    softmax reduction (over j) aligns with the matmul contraction axis and no
    transposes are ever needed.  exp() runs on ScalarE straight out of PSUM.
  - P@V gives resU^T [65, S] (row 64 = softmax denominators); normalization is
    a reciprocal of the sums row + gpsimd partition_broadcast + one DVE mul.
  - out-projection consumes resT directly; residual-add fused in the epilogue.
Attention/projection matmuls run in bf16 (fp32 PSUM accumulation); groupnorm
statistics stay fp32.  The softmax max-subtraction is skipped: scores are
~N(0,1) by construction (standardized activations, 1/sqrt(dk) folded into the
Q weights host-side), so exp() stays comfortably in fp32 range.
"""

import numpy as np
import ml_dtypes

import concourse.bacc as bacc
import concourse.tile as tile
from concourse import mybir
from concourse.bass_utils import run_bass_kernel_spmd

N_CORES = 8
B, C, H, W = 16, 512, 32, 32
S = H * W                      # 1024
BL = B // N_CORES              # 2 batches per core
NH, DK = 8, 64
NG = 32                        # groupnorm groups
GSZ = C // NG                  # 16 channels per group
EPS = 1e-5
F32 = mybir.dt.float32
BF16 = mybir.dt.bfloat16
F8 = mybir.dt.float8e4
AF = mybir.ActivationFunctionType
OP = mybir.AluOpType
NPBF16 = ml_dtypes.bfloat16
NPF8 = ml_dtypes.float8_e4m3

# test.py can flip these; results stashed in LAST.
TRACE = False
RECIP_MODE = "approx_sbuf"  # "approx_sbuf" | "plain"
LAST = {}


def _build(has_bqk, has_bv, has_outb, debug=False):
    nc = bacc.Bacc()
    dbg = {}
    if debug:
        for nm, shp in (
            ("dbg_xh", [C, S]),
            ("dbg_qt", [C, S]),
            ("dbg_kt", [C, S]),
            ("dbg_v", [8, 128, NH * 65]),
            ("dbg_ex", [128, S]),
            ("dbg_pvt", [65, S]),
            ("dbg_rrow", [1, S]),
            ("dbg_rbt", [64, S]),
            ("dbg_rt", [C, S]),
        ):
            dbg[nm] = nc.dram_tensor(nm, shp, F32, kind="ExternalOutput")

    x_d = nc.dram_tensor("x", [BL, C, S], F32, kind="ExternalInput")
    wqt_d = nc.dram_tensor("wqt", [C, C], BF16, kind="ExternalInput")   # [c_in, q_row]
    wkt_d = nc.dram_tensor("wkt", [C, C], BF16, kind="ExternalInput")
    wvt_d = nc.dram_tensor("wvt", [C, C], BF16, kind="ExternalInput")
    wot_d = nc.dram_tensor("wot", [C, C], BF16, kind="ExternalInput")   # [d_out, c_out]
    g_d = nc.dram_tensor("gmat", [128, 8], F32, kind="ExternalInput")
    gt_d = nc.dram_tensor("gtmat", [8, 128], F32, kind="ExternalInput")
    zpad_d = nc.dram_tensor("zpad", [64, S], BF16, kind="ExternalInput")
    bqk_d = (
        nc.dram_tensor("bqk", [128, 8], F32, kind="ExternalInput") if has_bqk else None
    )
    bv_d = nc.dram_tensor("bv", [1, C], BF16, kind="ExternalInput") if has_bv else None
    outb_d = (
        nc.dram_tensor("outb", [128, 4], F32, kind="ExternalInput") if has_outb else None
    )
    out_d = nc.dram_tensor("out", [BL, C, S], F32, kind="ExternalOutput")

    with tile.TileContext(nc) as tc:
        with (
            tc.tile_pool(name="const", bufs=1) as const,
            tc.tile_pool(name="px", bufs=4) as px,
            tc.tile_pool(name="pxe", bufs=4) as pxe,
            tc.tile_pool(name="pgn", bufs=4) as pgn,
            tc.tile_pool(name="pxh", bufs=8) as pxh,
            tc.tile_pool(name="pqt", bufs=8) as pqt,
            tc.tile_pool(name="pkt", bufs=16) as pkt,
            tc.tile_pool(name="pv", bufs=16) as pvp,
            tc.tile_pool(name="pexp", bufs=9) as pexp,
            tc.tile_pool(name="prec", bufs=3) as prec,
            tc.tile_pool(name="prt", bufs=8) as prt,
            tc.tile_pool(name="pout", bufs=3) as pout,
            tc.tile_pool(name="pps", bufs=2, space="PSUM") as pps,
            tc.tile_pool(name="psc", bufs=2, space="PSUM") as psc,
            tc.tile_pool(name="ppv", bufs=2, space="PSUM") as ppv,
        ):
            # ---- batch-0 x first (groupnorm needs it before weights)
            xt0 = []
            for cb in range(4):
                t = px.tile([128, S], F32, tag="x", name=f"x0_{cb}")
                nc.sync.dma_start(out=t, in_=x_d[0, cb * 128 : (cb + 1) * 128, :])
                xt0.append(t)

            # ---- constants into SBUF (small groupnorm mats first)
            g_sb = const.tile([128, 8], F32, tag="g")
            nc.sync.dma_start(out=g_sb, in_=g_d[:, :])
            gt_sb = const.tile([8, 128], F32, tag="gt")
            nc.sync.dma_start(out=gt_sb, in_=gt_d[:, :])
            wq_sb, wk_sb, wv_sb, wo_sb = [], [], [], []
            for nm, lst, src in (
                ("q", wq_sb, wqt_d),
                ("k", wk_sb, wkt_d),
                ("v", wv_sb, wvt_d),
                ("o", wo_sb, wot_d),
            ):
                for cb in range(4):
                    t = const.tile([128, C], BF16, tag=f"w_{nm}_{cb}")
                    nc.sync.dma_start(out=t, in_=src[cb * 128 : (cb + 1) * 128, :])
                    lst.append(t)
            eps_sb = const.tile([128, 1], F32, tag="eps")
            nc.vector.memset(eps_sb, EPS)
            nbias = const.tile([128, 1], F32, tag="nbias")
            nc.vector.memset(nbias, -1.0)
            if has_bqk:
                bqk_sb = const.tile([128, 8], F32, tag="bqk")
                nc.sync.dma_start(out=bqk_sb, in_=bqk_d[:, :])
            if has_bv:
                bv_sb = const.tile([1, C], BF16, tag="bv")
                nc.sync.dma_start(out=bv_sb, in_=bv_d[:, :])
                ones_sb = const.tile([1, S], BF16, tag="ones")
                nc.vector.memset(ones_sb, 1.0)
            if has_outb:
                outb_sb = const.tile([128, 4], F32, tag="outb")
                nc.sync.dma_start(out=outb_sb, in_=outb_d[:, :])

            # ---- PE warm-up: ~5us of dense dummy matmuls during the
            # DMA/groupnorm-bound startup so the HAM clock gate opens before
            # the first real projection matmul arrives.
            warm_ps = pps.tile([8, 128], F32, tag="pp", name="warm_ps")
            for wi in range(12):
                nc.tensor.matmul(
                    out=warm_ps,
                    lhsT=g_sb,
                    rhs=xt0[0][:, 0:128],
                    start=True,
                    stop=True,
                )

            # ================= emission helpers =================
            def load_x(b):
                xt = []
                for cb in range(4):
                    t = px.tile([128, S], F32, tag="x", name=f"x{b}_{cb}")
                    nc.sync.dma_start(out=t, in_=x_d[b, cb * 128 : (cb + 1) * 128, :])
                    xt.append(t)
                return xt

            def gn_batch(b, xt, xh):
                # groupnorm -> xhat for all 4 channel blocks; rstd computed on
                # DVE only (reciprocal seed + 2 Newton rsqrt steps) so ScalarE
                # never loads a non-Exp activation table.
                pgall = pps.tile([8, 4, 2], F32, tag="pp")   # [group, cb, (mean,e2)]
                for cb in range(4):
                    st6 = pgn.tile([128, 2, 6], F32, tag="st6")
                    nc.vector.bn_stats(out=st6[:, 0, :], in_=xt[cb][:, 0:512])
                    nc.vector.bn_stats(out=st6[:, 1, :], in_=xt[cb][:, 512:1024])
                    mv = pgn.tile([128, 2], F32, tag="mv")
                    nc.vector.bn_aggr(out=mv, in_=st6)
                    me2 = pgn.tile([128, 2], F32, tag="me2")
                    nc.vector.tensor_copy(out=me2[:, 0:1], in_=mv[:, 0:1])
                    nc.vector.tensor_tensor(
                        out=me2[:, 1:2], in0=mv[:, 0:1], in1=mv[:, 0:1], op=OP.mult
                    )
                    nc.vector.tensor_tensor(
                        out=me2[:, 1:2], in0=me2[:, 1:2], in1=mv[:, 1:2], op=OP.add
                    )
                    nc.tensor.matmul(
                        out=pgall[:, cb, :], lhsT=g_sb, rhs=me2, start=True, stop=True
                    )
                # group stats for all blocks at once ([8, 4] tiles)
                gm = pgn.tile([8, 4], F32, tag="gm")
                z = pgn.tile([8, 4], F32, tag="z")
                t2 = pgn.tile([8, 4], F32, tag="t2")
                y = pgn.tile([8, 4], F32, tag="y")
                nc.vector.tensor_scalar(
                    out=gm, in0=pgall[:, :, 0], scalar1=1.0 / GSZ, scalar2=None,
                    op0=OP.mult,
                )
                nc.vector.tensor_scalar(
                    out=z, in0=pgall[:, :, 1], scalar1=1.0 / GSZ, scalar2=EPS,
                    op0=OP.mult, op1=OP.add,
                )
                nc.vector.tensor_tensor(out=t2, in0=gm, in1=gm, op=OP.mult)
                nc.vector.tensor_tensor(out=z, in0=z, in1=t2, op=OP.subtract)
                # rsqrt(z): y0 = 1/z, then y <- y*(1.5 - 0.5*z*y^2) twice
                nc.vector.reciprocal(out=y, in_=z)
                for _ in range(2):
                    nc.vector.tensor_tensor(out=t2, in0=z, in1=y, op=OP.mult)
                    nc.vector.tensor_tensor(out=t2, in0=t2, in1=y, op=OP.mult)
                    nc.vector.tensor_scalar(
                        out=t2, in0=t2, scalar1=-0.5, scalar2=1.5,
                        op0=OP.mult, op1=OP.add,
                    )
                    nc.vector.tensor_tensor(out=y, in0=y, in1=t2, op=OP.mult)
                gs2 = pgn.tile([8, 2, 4], F32, tag="gs2")   # [(mean,rstd), cb]
                nc.vector.tensor_copy(out=gs2[:, 0, :], in_=gm)
                nc.vector.tensor_copy(out=gs2[:, 1, :], in_=y)
                for cb in range(4):
                    pb = pps.tile([128, 2], F32, tag="pp")
                    nc.tensor.matmul(
                        out=pb, lhsT=gt_sb, rhs=gs2[:, :, cb], start=True, stop=True
                    )
                    t = pxh.tile([128, S], BF16, tag="xh", name=f"xh{b}_{cb}")
                    nc.vector.tensor_scalar(
                        out=t,
                        in0=xt[cb],
                        scalar1=pb[:, 0:1],
                        scalar2=pb[:, 1:2],
                        op0=OP.subtract,
                        op1=OP.mult,
                    )
                    xh.append(t)
                    if debug and b == 0:
                        nc.gpsimd.dma_start(
                            out=dbg["dbg_xh"][cb * 128 : (cb + 1) * 128, :], in_=t
                        )

            def v_group(b, xh, vt, st):
                # one [S-tile, NH, 65] V tile with ones column per head
                pv = pps.tile([128, 512], F32, tag="pp")
                for cb in range(4):
                    nc.tensor.matmul(
                        out=pv,
                        lhsT=xh[cb][:, st * 128 : (st + 1) * 128],
                        rhs=wv_sb[cb],
                        start=(cb == 0),
                        stop=(cb == 3 and not has_bv),
                    )
                if has_bv:
                    nc.tensor.matmul(
                        out=pv,
                        lhsT=ones_sb[:, st * 128 : (st + 1) * 128],
                        rhs=bv_sb,
                        start=False,
                        stop=True,
                    )
                if st % 2 == 0:
                    t = pvp.tile([128, 2, NH, 72], F8, tag="v", name=f"v{b}_{st // 2}")
                    vt.append(t)
                else:
                    t = vt[st // 2]
                sl = st % 2
                nc.vector.memset(t[:, sl, :, 64:65], 1.0)
                nc.vector.tensor_copy(
                    out=t[:, sl, :, 0:64], in_=pv.rearrange("p (h d) -> p h d", h=NH)
                )
                if debug and b == 0:
                    nc.gpsimd.dma_start(
                        out=dbg["dbg_v"][st, :, :],
                        in_=t.rearrange("p h d -> p (h d)"),
                    )

            def attn_head(b, qt, kt, vt, rt, h, fill=None, fill_every=2):
                # scores transposed -> exp -> P@V halves -> normalized resT rows
                # `fill` emits one unit of independent PE work per jb so the
                # Tensor engine stays dense while ScalarE works through exps.
                hp, off = h // 2, (h % 2) * 64
                ex_tiles = [None] * 4
                for jb in range(8):
                    ps = psc.tile([128, S], F32, tag="ps")
                    for sc in range(2):
                        cols = slice(sc * 512, (sc + 1) * 512)
                        nc.tensor.matmul(
                            out=ps[:, cols],
                            lhsT=kt[h][:, jb * 128 : (jb + 1) * 128],
                            rhs=qt[hp][:, cols],
                            start=True,
                            stop=True,
                        )
                    pair, sl = jb // 2, jb % 2
                    if sl == 0:
                        ex_tiles[pair] = pexp.tile(
                            [128, 2, S], F8, tag="ex", name=f"ex{b}_{h}_{pair}"
                        )
                    # exp(x - 1): bias keeps e4m3 P values well under the 448
                    # max; the uniform e^-1 cancels in the normalization.
                    nc.scalar.activation(
                        out=ex_tiles[pair][:, sl, :], in_=ps, func=AF.Exp,
                        bias=nbias[:, 0:1],
                    )
                    if fill is not None and jb % fill_every == fill_every - 1:
                        fill()

                # P@V: fp8 DoubleRow, two j-blocks per instruction (2x rate);
                # jb-pair-major so both column halves reuse each weight load
                pvts = [ppv.tile([65, 512], F32, tag="ppvt", name=f"pvt{i}") for i in range(2)]
                for pair in range(4):
                    for sc in range(2):
                        nc.tensor.matmul(
                            out=pvts[sc],
                            lhsT=vt[pair][:, :, h, 0:65],
                            rhs=ex_tiles[pair][:, :, sc * 512 : (sc + 1) * 512],
                            start=(pair == 0),
                            stop=(pair == 3),
                            perf_mode=mybir.MatmulPerfMode.DoubleRow,
                        )
                for sc in range(2):
                    pvt = pvts[sc]
                    if debug and b == 0 and h == 0:
                        dump = pout.tile([65, 512], F32, tag="dump", name=f"dmp{sc}")
                        nc.vector.tensor_copy(out=dump, in_=pvt)
                        nc.gpsimd.dma_start(
                            out=dbg["dbg_pvt"][:, sc * 512 : (sc + 1) * 512],
                            in_=dump,
                        )
                    # normalize: resT[head rows] = resU / rowsum
                    rrow = prec.tile([1, 512], F32, tag="rr")
                    if RECIP_MODE == "approx_sbuf":
                        stage = prec.tile([1, 512], F32, tag="st")
                        nc.vector.tensor_copy(out=stage, in_=pvt[64:65, :])
                        nc.vector.reciprocal_approx_fast(out=rrow, in_=stage)
                    else:
                        nc.vector.reciprocal(out=rrow, in_=pvt[64:65, :])
                    rbt = prec.tile([64, 512], F32, tag="rb")
                    nc.gpsimd.partition_broadcast(rbt, rrow)
                    nc.vector.tensor_tensor(
                        out=rt[hp][off : off + 64, sc * 512 : (sc + 1) * 512],
                        in0=pvt[0:64, :],
                        in1=rbt,
                        op=OP.mult,
                    )
                    if debug and b == 0 and h == 0:
                        nc.gpsimd.dma_start(
                            out=dbg["dbg_rrow"][:, sc * 512 : (sc + 1) * 512],
                            in_=rrow,
                        )
                        nc.gpsimd.dma_start(
                            out=dbg["dbg_rbt"][:, sc * 512 : (sc + 1) * 512],
                            in_=rbt,
                        )

            def drain(wl, n):
                for _ in range(min(n, len(wl))):
                    wl.pop(0)()

            def qk_units(b, xh, dst, w_sb, boff, rb):
                # one projection psum row-block.  For Q (boff==0): a single
                # head-pair tile.  For K (boff==4): two per-head tiles with the
                # other head's 64 rows zeroed, so the scores matmul can run a
                # full-K (128-partition) contraction at full SBUF stream rate.
                holder = {}

                def half(sc):
                    is_q = boff == 0
                    if "t" not in holder:
                        if is_q:
                            tq = pqt.tile(
                                [128, S], BF16, tag="qk", name=f"q{b}_{rb}"
                            )
                            holder["t"] = (tq,)
                            dst.append(tq)
                        else:
                            te = pkt.tile(
                                [128, S], BF16, tag="qk", name=f"k{b}_{rb}e"
                            )
                            to = pkt.tile(
                                [128, S], BF16, tag="qk", name=f"k{b}_{rb}o"
                            )
                            nc.sync.dma_start(out=te[64:128, :], in_=zpad_d[:, :])
                            nc.sync.dma_start(out=to[0:64, :], in_=zpad_d[:, :])
                            holder["t"] = (te, to)
                            dst.extend([te, to])
                    tiles = holder["t"]
                    pq = pps.tile([128, 512], F32, tag="pp")
                    for cb in range(4):
                        nc.tensor.matmul(
                            out=pq,
                            lhsT=w_sb[cb][:, rb * 128 : (rb + 1) * 128],
                            rhs=xh[cb][:, sc * 512 : (sc + 1) * 512],
                            start=(cb == 0),
                            stop=(cb == 3),
                        )
                    cols = slice(sc * 512, (sc + 1) * 512)
                    if is_q:
                        if has_bqk:
                            nc.vector.tensor_scalar_add(
                                out=tiles[0][:, cols],
                                in0=pq,
                                scalar1=bqk_sb[:, rb : rb + 1],
                            )
                        else:
                            nc.vector.tensor_copy(out=tiles[0][:, cols], in_=pq)
                    else:
                        for t, prng in (
                            (tiles[0], slice(0, 64)),
                            (tiles[1], slice(64, 128)),
                        ):
                            if has_bqk:
                                nc.vector.tensor_scalar_add(
                                    out=t[prng, cols],
                                    in0=pq[prng, :],
                                    scalar1=bqk_sb[prng, 4 + rb : 5 + rb],
                                )
                            else:
                                nc.vector.tensor_copy(out=t[prng, cols], in_=pq[prng, :])
                    if debug and b == 0 and sc == 1:
                        if is_q:
                            nc.gpsimd.dma_start(
                                out=dbg["dbg_qt"][rb * 128 : (rb + 1) * 128, :],
                                in_=tiles[0],
                            )
                        else:
                            nc.gpsimd.dma_start(
                                out=dbg["dbg_kt"][rb * 128 : rb * 128 + 64, :],
                                in_=tiles[0][0:64, :],
                            )
                            nc.gpsimd.dma_start(
                                out=dbg["dbg_kt"][rb * 128 + 64 : (rb + 1) * 128, :],
                                in_=tiles[1][64:128, :],
                            )

                return [lambda: half(0), lambda: half(1)]

            def epi_units(b, rt, cb):
                # epi_block split into two 512-column half-units (DMA on 2nd);
                # the residual x slice is re-loaded from DRAM.
                holder = {}

                def half(sc):
                    if "t" not in holder:
                        holder["t"] = pout.tile(
                            [128, S], F32, tag="ot", name=f"ot{b}_{cb}"
                        )
                    ot = holder["t"]
                    xre = pxe.tile([128, 512], F32, tag="xe")
                    nc.sync.dma_start(
                        out=xre,
                        in_=x_d[b, cb * 128 : (cb + 1) * 128, sc * 512 : (sc + 1) * 512],
                    )
                    po = pps.tile([128, 512], F32, tag="pp")
                    for db in range(4):
                        nc.tensor.matmul(
                            out=po,
                            lhsT=wo_sb[db][:, cb * 128 : (cb + 1) * 128],
                            rhs=rt[db][:, sc * 512 : (sc + 1) * 512],
                            start=(db == 0),
                            stop=(db == 3),
                        )
                    dst_ap = ot[:, sc * 512 : (sc + 1) * 512]
                    if has_outb:
                        nc.vector.scalar_tensor_tensor(
                            out=dst_ap,
                            in0=po,
                            scalar=outb_sb[:, cb : cb + 1],
                            in1=xre,
                            op0=OP.add,
                            op1=OP.add,
                        )
                    else:
                        nc.vector.tensor_tensor(out=dst_ap, in0=po, in1=xre, op=OP.add)
                    if sc == 1:
                        nc.sync.dma_start(
                            out=out_d[b, cb * 128 : (cb + 1) * 128, :], in_=ot
                        )

                return [lambda: half(0), lambda: half(1)]

            # ================= schedule =================
            # batch 0 prep emitted directly; batch 1 prep + batch 0 epilogue
            # are emitted one psum-group at a time inside the attention loops
            # (fill callback per jb) so the Tensor engine always has dense
            # independent work while ScalarE works through the exps.
            xt1 = load_x(1)
            xh0, qt0, kt0, vt0 = [], [], [], []
            gn_batch(0, xt0, xh0)
            # V first, then only the rb0 row-blocks of Q/K: attention(0) can
            # then start at head 0 while the remaining row-blocks stream in as
            # fill work inside the head loop.
            for st in range(8):
                v_group(0, xh0, vt0, st)
            for u in qk_units(0, xh0, qt0, wq_sb, 0, 0):
                u()
            for u in qk_units(0, xh0, kt0, wk_sb, 4, 0):
                u()

            xh1, qt1, kt1, vt1 = [], [], [], []
            gn_batch(1, xt1, xh1)
            work1 = []
            for rb in range(1, 4):
                work1.extend(qk_units(0, xh0, qt0, wq_sb, 0, rb))
                work1.extend(qk_units(0, xh0, kt0, wk_sb, 4, rb))
            for rb in range(2):
                work1.extend(qk_units(1, xh1, qt1, wq_sb, 0, rb))
                work1.extend(qk_units(1, xh1, kt1, wk_sb, 4, rb))
            for st in range(8):
                work1.append(lambda st=st: v_group(1, xh1, vt1, st))

            rt0 = [prt.tile([128, S], BF16, tag="rt", name=f"rt0_{i}") for i in range(4)]
            for h in range(NH):
                attn_head(0, qt0, kt0, vt0, rt0, h, fill=lambda: drain(work1, 1))
            drain(work1, len(work1))
            if debug:
                for hp in range(4):
                    nc.gpsimd.dma_start(
                        out=dbg["dbg_rt"][hp * 128 : (hp + 1) * 128, :], in_=rt0[hp]
                    )

            # batch-1 attention: deferred qk row-blocks (heads 4-7) first, then
            # batch-0 epilogue halves, spread evenly (fill every 4th jb)
            work2 = []
            for rb in range(2, 4):
                work2.extend(qk_units(1, xh1, qt1, wq_sb, 0, rb))
                work2.extend(qk_units(1, xh1, kt1, wk_sb, 4, rb))
            for cb in range(4):
                work2.extend(epi_units(0, rt0, cb))
            rt1 = [prt.tile([128, S], BF16, tag="rt", name=f"rt1_{i}") for i in range(4)]
            for h in range(NH):
                attn_head(
                    1, qt1, kt1, vt1, rt1, h,
                    fill=lambda: drain(work2, 1), fill_every=4,
                )
            drain(work2, len(work2))
            for cb in range(4):
                for u in epi_units(1, rt1, cb):
                    u()

    nc.finalize()
    return nc


def kernel(**inputs):
    x = np.asarray(inputs["x"], np.float32)
    norm_w = np.asarray(inputs["norm_w"], np.float64)
    norm_b = np.asarray(inputs["norm_b"], np.float64)
    proj_w = np.asarray(inputs["proj_w"], np.float64)
    proj_b = np.asarray(inputs["proj_b"], np.float64)
    out_w = np.asarray(inputs["out_w"], np.float32)
    out_b = np.asarray(inputs["out_b"], np.float32)

    # split qkv rows (row = h*192 + t*64 + d, t in {q,k,v}) into head-major mats
    pw = proj_w.reshape(NH, 3, DK, C)
    pb = proj_b.reshape(NH, 3, DK)
    mats, biases = [], []
    for t in range(3):
        wm = pw[:, t].reshape(NH * DK, C)
        bv = pb[:, t].reshape(NH * DK)
        # fold groupnorm affine: y = xhat*nw + nb  =>  W@y + b = (W*nw)@xhat + (W@nb + b)
        mats.append(wm * norm_w[None, :])
        biases.append(bv + wm @ norm_b)
    wq, wk, wv = mats
    bq, bk, bv = biases
    scale = DK ** -0.5
    wq = wq * scale
    bq = bq * scale

    wqT = np.ascontiguousarray(wq.T).astype(NPBF16)
    wkT = np.ascontiguousarray(wk.T).astype(NPBF16)
    wvT = np.ascontiguousarray(wv.T * 4.0).astype(NPBF16)
    woT = np.ascontiguousarray(out_w.T / 4.0).astype(NPBF16)

    G = np.zeros((128, 8), np.float32)
    G[np.arange(128), np.arange(128) // GSZ] = 1.0
    GT = np.ascontiguousarray(G.T)
    ZPAD = np.zeros((64, S), NPBF16)

    has_bqk = bool(np.any(bq) or np.any(bk))
    has_bv = bool(np.any(bv))
    has_outb = bool(np.any(out_b))

    bqk = np.zeros((128, 8), np.float32)
    bqk[:, 0:4] = bq.reshape(4, 128).T
    bqk[:, 4:8] = bk.reshape(4, 128).T
    outb128 = np.ascontiguousarray(out_b.reshape(4, 128).T)

    nc = _build(has_bqk, has_bv, has_outb)

    xr = x.reshape(B, C, S)
    in_maps = []
    for c in range(N_CORES):
        m = {
            "x": np.ascontiguousarray(xr[c * BL : (c + 1) * BL]),
            "wqt": wqT,
            "wkt": wkT,
            "wvt": wvT,
            "wot": woT,
            "gmat": G,
            "gtmat": GT,
            "zpad": ZPAD,
        }
        if has_bqk:
            m["bqk"] = bqk
        if has_bv:
            m["bv"] = np.ascontiguousarray(bv.reshape(1, C)).astype(NPBF16)
        if has_outb:
            m["outb"] = outb128
        in_maps.append(m)

    # guard: bass_utils imports antenv.axon_hooks when tracing is requested
    # (e.g. via BASS_TRACE env); provide a no-op module if the image lacks it.
    try:
        import antenv.axon_hooks  # noqa: F401
    except ImportError:
        import sys
        import types

        import antenv

        _m = types.ModuleType("antenv.axon_hooks")
        _m._hook = None
        _m.set_axon_ntff_profile_hook = lambda h: setattr(_m, "_hook", h)
        _m.get_axon_ntff_profile_hook = lambda: _m._hook
        sys.modules["antenv.axon_hooks"] = _m
        antenv.axon_hooks = _m

    res = None
    for attempt in range(3):
        try:
            res = run_bass_kernel_spmd(
                nc, in_maps, core_ids=list(range(N_CORES)), trace=TRACE
            )
            break
        except Exception:
            # transient NRT_EXEC_UNIT_UNRECOVERABLE-style device hiccups
            # clear on retry; re-raise on the final attempt
            if attempt == 2:
                raise
    LAST["exec_time_ns"] = res.exec_time_ns
    LAST["mean_exec_time_ns"] = res.mean_exec_time_ns
    LAST["result"] = res

    out = np.concatenate([res.results[c]["out"] for c in range(N_CORES)], axis=0)
    return np.ascontiguousarray(out.reshape(B, C, H, W).astype(np.float32))

